# revision 7
# baseline (speedup 1.0000x reference)
"""Llama MHA layer on 8 TRN2 NeuronCores — v2 (bf16, SBUF-resident).

Causal-balanced sequence sharding, no collectives. Core c owns chunk A
(small prefix) and chunk B (large prefix) from the two batches. Column
layout per core (3072 cols):
  [ownA 0:256 | ownB 256:512 | prefA+pad 512:1280 | prefB+pad 1280:3072]
K/V are recomputed locally for prefixes; padding cols are zero and the
exp-bias input (-1e30 for pad key-tiles) zeroes their softmax weight.

All matmuls run in bf16 (PE full rate); norm weights are folded into the
projection weights host-side; weights are host-packed so every weight
DMA is one large contiguous transfer. K/V/Q/ctx stay SBUF-resident.
Scores use 64-partition row-tiled matmul pairs (two kv-head strips run
concurrently); softmax denominators broadcast via a ones-column matmul
instead of DRAM bounces.
"""

import numpy as np
import ml_dtypes

BF = ml_dtypes.bfloat16

D_MODEL = 2048
N_HEADS = 32
N_KV = 8
HEAD_DIM = 64
D_FF = 8192
ROPE_BASE = 10000.0
EPS = 1e-5
B, S = 2, 2048
CHUNK = 256
P = 128
KD = 16           # d_model k-tiles
N_CORES = 8
NCOL = 3072       # padded kv columns per core
NEG = -1e30
BLK = 512
NBLK = NCOL // BLK   # 6
ATT_SCALE = 1.0 / np.sqrt(HEAD_DIM)
INV_D = 1.0 / D_MODEL

# kt tile lists per query chunk (tile index = col/128)
KTL_A = [0, 1, 4, 5, 6, 7, 8, 9]
KTL_B = [2, 3] + list(range(10, 24))

_prog_cache = {}


def _build_program():
    import concourse.bacc as bacc
    import concourse.bass as bass
    import concourse.mybir as mybir
    import concourse.tile as tile

    F32 = mybir.dt.float32
    F32R = mybir.dt.float32r
    BF16 = mybir.dt.bfloat16
    AF = mybir.ActivationFunctionType

    nc = bacc.Bacc(None, target_bir_lowering=False)

    # ---- inputs -------------------------------------------------------
    xT = nc.dram_tensor("xT", [D_MODEL, NCOL], BF16, kind="ExternalInput")
    xo32 = nc.dram_tensor("xo32", [D_MODEL, 512], F32, kind="ExternalInput")
    cosT = nc.dram_tensor("cosT", [P, NCOL], BF16, kind="ExternalInput")
    sinT = nc.dram_tensor("sinT", [P, NCOL], BF16, kind="ExternalInput")
    mask2 = nc.dram_tensor("mask2", [P, 2, 2, 256], BF16, kind="ExternalInput")
    ebias = nc.dram_tensor("ebias", [P, 2, 16], F32, kind="ExternalInput")
    w_q = nc.dram_tensor("w_q", [16, P, KD, P], BF16, kind="ExternalInput")
    w_k = nc.dram_tensor("w_k", [P, KD, 512], BF16, kind="ExternalInput")
    w_v = nc.dram_tensor("w_v", [P, KD, 512], BF16, kind="ExternalInput")
    w_o = nc.dram_tensor("w_o", [16, P, KD, P], BF16, kind="ExternalInput")
    w_g = nc.dram_tensor("w_g", [64, P, KD, P], BF16, kind="ExternalInput")
    w_u = nc.dram_tensor("w_u", [64, P, KD, P], BF16, kind="ExternalInput")
    w_d = nc.dram_tensor("w_d", [16, P, 64, P], BF16, kind="ExternalInput")
    permM = nc.dram_tensor("permM", [P, P], BF16, kind="ExternalInput")
    onesC = nc.dram_tensor("onesC", [P, 1], BF16, kind="ExternalInput")
    onesR = nc.dram_tensor("onesR", [1, P], BF16, kind="ExternalInput")
    outT = nc.dram_tensor("outT", [D_MODEL, 512], F32, kind="ExternalOutput")

    _name_ctr = [0]

    def _nm(tag):
        _name_ctr[0] += 1
        return f"{tag}_{_name_ctr[0]}"

    with tile.TileContext(nc) as tc:
        import contextlib
        stack = contextlib.ExitStack()
        with stack:
            const = stack.enter_context(tc.tile_pool(name="const", bufs=1))

            ones_sb = const.tile([P, 1], BF16, tag="ones", name=_nm("ones"))
            nc.sync.dma_start(out=ones_sb, in_=onesC.ap())
            onesr_sb = const.tile([1, P], BF16, tag="onesr", name=_nm("onesr"))
            nc.sync.dma_start(out=onesr_sb, in_=onesR.ap())
            perm_sb = const.tile([P, P], BF16, tag="perm", name=_nm("perm"))
            nc.sync.dma_start(out=perm_sb, in_=permM.ap())
            mask_sb = const.tile([P, 2, 2, 256], BF16, tag="mask", name=_nm("mask"))
            nc.sync.dma_start(out=mask_sb, in_=mask2.ap())
            eb_sb = const.tile([P, 2, 16], F32, tag="eb", name=_nm("eb"))
            nc.sync.dma_start(out=eb_sb, in_=ebias.ap())
            eps_sb = const.tile([1, 1], F32, tag="eps", name=_nm("eps"))
            nc.vector.memset(eps_sb, EPS)

            # persistent across C/D
            big = stack.enter_context(tc.tile_pool(name="big", bufs=1))
            ctxt = big.tile([P, KD, 512], BF16, tag="ctx", name=_nm("ctx"))

            # K/V/Q live only through phases A+B
            kv_stack = contextlib.ExitStack()
            akv = kv_stack.enter_context(tc.tile_pool(name="akv", bufs=1))
            KT_sb = akv.tile([P, 4, NCOL], BF16, tag="KT", name=_nm("KT"))
            V_sb = akv.tile([P, 24, 8, 65], BF16, tag="V", name=_nm("V"))
            QT_sb = akv.tile([P, KD, 512], BF16, tag="QT", name=_nm("QT"))

            # gate columns (denominator accumulators) = 1.0
            nc.vector.memset(V_sb[:, :, :, 64:65], 1.0)

            # =========== PHASE A: rmsnorm1 + QKV proj + rope ===========
            with contextlib.ExitStack() as pa:
                wkv_p = pa.enter_context(tc.tile_pool(name="wkv", bufs=1))
                xin_p = pa.enter_context(tc.tile_pool(name="xin", bufs=1))
                xwn_p = pa.enter_context(tc.tile_pool(name="xwn", bufs=2))
                wq_p = pa.enter_context(tc.tile_pool(name="wqA", bufs=2))
                tab_p = pa.enter_context(tc.tile_pool(name="tabA", bufs=2))
                tmp_p = pa.enter_context(tc.tile_pool(name="tmpA", bufs=2))
                rop_p = pa.enter_context(tc.tile_pool(name="ropA", bufs=3))
                ps_ss = pa.enter_context(
                    tc.tile_pool(name="psss", bufs=1, space="PSUM"))
                ps_mm = pa.enter_context(
                    tc.tile_pool(name="psmm", bufs=5, space="PSUM"))
                ps_rp = pa.enter_context(
                    tc.tile_pool(name="psrp", bufs=1, space="PSUM"))

                wk_sb = wkv_p.tile([P, KD, 512], BF16, tag="wk", name=_nm("wk"))
                wv_sb = wkv_p.tile([P, KD, 512], BF16, tag="wv", name=_nm("wv"))

                def rope_write(psum, cs, sn, dst_ap):
                    # psum [128, 512] fp32 -> rope -> bf16 dst
                    raw = rop_p.tile([P, 512], BF16, tag="rraw", name=_nm("rraw"))
                    nc.scalar.activation(raw[:], psum, AF.Copy)
                    rot = ps_rp.tile([P, 512], F32, tag="rot", name=_nm("rot"))
                    nc.tensor.matmul(rot[:], perm_sb[:], raw[:],
                                     start=True, stop=True)
                    t1 = rop_p.tile([P, 512], BF16, tag="t1", name=_nm("t1"))
                    nc.vector.tensor_mul(t1[:], raw[:], cs[:])
                    t2 = rop_p.tile([P, 512], BF16, tag="t2", name=_nm("t2"))
                    nc.vector.tensor_mul(t2[:], rot[:], sn[:])
                    nc.vector.tensor_add(dst_ap, t1[:], t2[:])

                def stats_mm(b):
                    c0 = b * BLK
                    cs = tab_p.tile([P, BLK], BF16, tag="cos", name=_nm("cos"))
                    nc.sync.dma_start(out=cs, in_=cosT.ap()[:, c0:c0 + BLK])
                    sn = tab_p.tile([P, BLK], BF16, tag="sin", name=_nm("sin"))
                    nc.sync.dma_start(out=sn, in_=sinT.ap()[:, c0:c0 + BLK])
                    xin = xin_p.tile([P, KD, BLK], BF16, tag="xin",
                                     name=_nm("xin"))
                    for k in range(KD):
                        nc.sync.dma_start(
                            out=xin[:, k, :],
                            in_=xT.ap()[k * P:(k + 1) * P, c0:c0 + BLK])
                    if b == 0:
                        nc.sync.dma_start(out=wk_sb, in_=w_k.ap())
                        nc.sync.dma_start(out=wv_sb, in_=w_v.ap())
                    ssum = ps_ss.tile([1, BLK], F32, tag="ss", name=_nm("ss"))
                    for k in range(KD):
                        sq = tmp_p.tile([P, BLK], BF16, tag="sq", name=_nm("sq"))
                        nc.vector.tensor_mul(sq[:], xin[:, k, :], xin[:, k, :])
                        nc.tensor.matmul(ssum[:], ones_sb[:], sq[:],
                                         start=(k == 0), stop=(k == KD - 1))
                    return cs, sn, xin, ssum

                def stats_tail(pre):
                    cs, sn, xin, ssum = pre
                    std = tmp_p.tile([1, BLK], F32, tag="std", name=_nm("std"))
                    nc.scalar.activation(std[:], ssum[:], AF.Sqrt,
                                         bias=eps_sb[0:1, :], scale=INV_D)
                    inv32 = tmp_p.tile([1, BLK], F32, tag="inv32", name=_nm("inv32"))
                    nc.vector.reciprocal(inv32[:], std[:])
                    inv = tmp_p.tile([1, BLK], BF16, tag="inv", name=_nm("inv"))
                    nc.scalar.activation(inv[:], inv32[:], AF.Copy)
                    ibc = ps_ss.tile([P, BLK], F32, tag="ibc", name=_nm("ibc"))
                    nc.tensor.matmul(ibc[:], onesr_sb[:], inv[:],
                                     start=True, stop=True)
                    ibs = tmp_p.tile([P, BLK], BF16, tag="ibs", name=_nm("ibs"))
                    nc.scalar.activation(ibs[:], ibc[:], AF.Copy)
                    xwn = xwn_p.tile([P, KD, BLK], BF16, tag="xwn",
                                     name=_nm("xwn"))
                    for k in range(KD):
                        nc.vector.tensor_mul(xwn[:, k, :], xin[:, k, :], ibs[:])
                    return cs, sn, xwn

                st = stats_tail(stats_mm(0))
                for b in range(NBLK):
                    c0 = b * BLK
                    cs, sn, xwn = st

                    # K projection + rope -> KT_sb
                    for gp in range(4):
                        kps = ps_mm.tile([P, BLK], F32, tag="mm", name=_nm("mm"))
                        for k in range(KD):
                            nc.tensor.matmul(
                                kps[:], wk_sb[:, k, gp * P:(gp + 1) * P],
                                xwn[:, k, :],
                                start=(k == 0), stop=(k == KD - 1))
                        rope_write(kps[:], cs, sn, KT_sb[:, gp, c0:c0 + BLK])

                    # next block's rmsnorm stats: sums after K-proj (input
                    # DMAs have landed), serial tail before V-proj's last
                    # tile so nothing is exposed at the block boundary
                    pre = stats_mm(b + 1) if b + 1 < NBLK else None

                    # V projection -> V_sb (strided 65-col head slots)
                    for tm in range(4):
                        kt = b * 4 + tm
                        if tm == 3 and pre is not None:
                            st = stats_tail(pre)
                        vps = ps_mm.tile([P, 8, 64], F32, tag="mm", name=_nm("mm"))
                        for k in range(KD):
                            nc.tensor.matmul(
                                vps[:], xwn[:, k, tm * P:(tm + 1) * P],
                                wv_sb[:, k, :],
                                start=(k == 0), stop=(k == KD - 1))
                        nc.scalar.activation(
                            V_sb[:, kt, :, 0:64], vps[:], AF.Copy)

                    # Q projection + rope (own cols live in block 0)
                    if b == 0:
                        for t in range(KD):
                            wq_t = wq_p.tile([P, KD, P], BF16, tag="wq",
                                             name=_nm("wq"))
                            nc.sync.dma_start(out=wq_t, in_=w_q.ap()[t])
                            qps = ps_mm.tile([P, BLK], F32, tag="mm",
                                             name=_nm("mm"))
                            for k in range(KD):
                                nc.tensor.matmul(
                                    qps[:], wq_t[:, k, :], xwn[:, k, :],
                                    start=(k == 0), stop=(k == KD - 1))
                            rope_write(qps[:], cs, sn, QT_sb[:, t, :])

            # =========== PHASE B: attention ===========
            with contextlib.ExitStack() as pb:
                ex_p = pb.enter_context(tc.tile_pool(name="exB", bufs=16))
                sm_p = pb.enter_context(tc.tile_pool(name="smB", bufs=8))
                ps_sc = pb.enter_context(
                    tc.tile_pool(name="pssc", bufs=4, space="PSUM"))
                ps_cx = pb.enter_context(
                    tc.tile_pool(name="pscx", bufs=4, space="PSUM"))

                for cc in range(2):
                    ktl = KTL_A if cc == 0 else KTL_B
                    nkt = len(ktl)
                    qc = cc * 256
                    for gp in range(4):
                        cxs = [ps_cx.tile([65, 2, 256], F32, tag="cx",
                                          name=_nm("cx"))
                               for _ in range(4)]  # idx = half*2 + jp
                        pend = None  # (kti, kt, [ex x4]) awaiting AV
                        for kti, kt in enumerate(ktl):
                            exes = []
                            for half in range(2):
                                h0 = half * 64
                                for jp in range(2):
                                    t0 = gp * 4 + 2 * jp
                                    scp = ps_sc.tile([P, 2, 256], F32, tag="sc",
                                                     name=_nm("sc"))
                                    nc.tensor.matmul(
                                        scp[:],
                                        KT_sb[h0:h0 + 64, gp,
                                              kt * P:(kt + 1) * P],
                                        QT_sb[h0:h0 + 64, t0:t0 + 2,
                                              qc:qc + 256],
                                        start=True, stop=True)
                                    if kti < 2:
                                        nc.vector.tensor_add(
                                            scp[:], scp[:], mask_sb[:, kti])
                                    ex = ex_p.tile([P, 2, 256], BF16, tag="ex",
                                                   name=_nm("ex"))
                                    nc.scalar.activation(
                                        ex[:], scp[:], AF.Exp,
                                        bias=eb_sb[:, cc, kti:kti + 1],
                                        scale=ATT_SCALE)
                                    exes.append(ex)
                            if pend is not None:
                                pkti, pkt, pexes = pend
                                for half in range(2):
                                    for jp in range(2):
                                        nc.tensor.matmul(
                                            cxs[half * 2 + jp][:],
                                            V_sb[:, pkt, 2 * gp + half, :],
                                            pexes[half * 2 + jp][:],
                                            start=(pkti == 0),
                                            stop=(pkti == nkt - 1))
                            pend = (kti, kt, exes)
                        pkti, pkt, pexes = pend
                        for half in range(2):
                            for jp in range(2):
                                nc.tensor.matmul(
                                    cxs[half * 2 + jp][:],
                                    V_sb[:, pkt, 2 * gp + half, :],
                                    pexes[half * 2 + jp][:],
                                    start=(pkti == 0),
                                    stop=(pkti == nkt - 1))
                        for half in range(2):
                            h0 = half * 64
                            for jp in range(2):
                                t0 = gp * 4 + 2 * jp
                                cx = cxs[half * 2 + jp]
                                rec = sm_p.tile([1, 2, 256], F32, tag="rec",
                                                name=_nm("rec"))
                                nc.vector.reciprocal(rec[:], cx[64:65])
                                bcs = sm_p.tile([64, 2, 256], F32, tag="bcs",
                                                name=_nm("bcs"))
                                nc.gpsimd.partition_broadcast(bcs[:], rec[:])
                                nc.vector.tensor_mul(
                                    ctxt[h0:h0 + 64, t0:t0 + 2, qc:qc + 256],
                                    cx[0:64], bcs[:])
            kv_stack.close()

            # =========== PHASE C: out-proj + residual + rmsnorm2 =======
            res_p = stack.enter_context(tc.tile_pool(name="res", bufs=1))
            yT = res_p.tile([P, KD, 512], F32, tag="yT", name=_nm("yT"))
            h2 = res_p.tile([P, KD, 512], BF16, tag="h2", name=_nm("h2"))
            with contextlib.ExitStack() as pc:
                xo_p = pc.enter_context(tc.tile_pool(name="xoC", bufs=1))
                wo_p = pc.enter_context(tc.tile_pool(name="woC", bufs=6))
                tmp2_p = pc.enter_context(tc.tile_pool(name="tmpC", bufs=3))
                ps_y = pc.enter_context(
                    tc.tile_pool(name="psy", bufs=3, space="PSUM"))
                ps_s2 = pc.enter_context(
                    tc.tile_pool(name="pss2", bufs=1, space="PSUM"))

                xo = xo_p.tile([P, KD, 512], F32, tag="xo", name=_nm("xo"))
                for k in range(KD):
                    nc.sync.dma_start(
                        out=xo[:, k, :], in_=xo32.ap()[k * P:(k + 1) * P, :])

                for m in range(KD):
                    wo_t = wo_p.tile([P, KD, P], BF16, tag="wo", name=_nm("wo"))
                    nc.sync.dma_start(out=wo_t, in_=w_o.ap()[m])
                    yps = ps_y.tile([P, 512], F32, tag="y", name=_nm("y"))
                    for t in range(KD):
                        nc.tensor.matmul(yps[:], wo_t[:, t, :], ctxt[:, t, :],
                                         start=(t == 0), stop=(t == KD - 1))
                    nc.vector.tensor_add(yT[:, m, :], yps[:], xo[:, m, :])

                ss2 = ps_s2.tile([1, 512], F32, tag="ss2", name=_nm("ss2"))
                for m in range(KD):
                    sq2 = tmp2_p.tile([P, 512], BF16, tag="sq2", name=_nm("sq2"))
                    nc.vector.tensor_mul(sq2[:], yT[:, m, :], yT[:, m, :])
                    nc.tensor.matmul(ss2[:], ones_sb[:], sq2[:],
                                     start=(m == 0), stop=(m == KD - 1))
                std2 = tmp2_p.tile([1, 512], F32, tag="std2", name=_nm("std2"))
                nc.scalar.activation(std2[:], ss2[:], AF.Sqrt,
                                     bias=eps_sb[0:1, :], scale=INV_D)
                inv2f = tmp2_p.tile([1, 512], F32, tag="inv2f", name=_nm("inv2f"))
                nc.vector.reciprocal(inv2f[:], std2[:])
                inv2 = tmp2_p.tile([1, 512], BF16, tag="inv2", name=_nm("inv2"))
                nc.scalar.activation(inv2[:], inv2f[:], AF.Copy)
                ibc2 = ps_s2.tile([P, 512], F32, tag="ibc2", name=_nm("ibc2"))
                nc.tensor.matmul(ibc2[:], onesr_sb[:], inv2[:],
                                 start=True, stop=True)
                ibs2 = tmp2_p.tile([P, 512], BF16, tag="ibs2", name=_nm("ibs2"))
                nc.scalar.activation(ibs2[:], ibc2[:], AF.Copy)
                for m in range(KD):
                    nc.vector.tensor_mul(h2[:, m, :], yT[:, m, :], ibs2[:])

            # =========== PHASE D: SwiGLU MLP ===========
            with contextlib.ExitStack() as pd:
                ht_p = pd.enter_context(tc.tile_pool(name="htD", bufs=2))
                y2_p = pd.enter_context(tc.tile_pool(name="y2D", bufs=1))
                wgu_p = pd.enter_context(tc.tile_pool(name="wguD", bufs=6))
                wd_p = pd.enter_context(tc.tile_pool(name="wdD", bufs=4))
                tmp3_p = pd.enter_context(tc.tile_pool(name="tmpD", bufs=4))
                ps_gu = pd.enter_context(
                    tc.tile_pool(name="psgu", bufs=4, space="PSUM"))
                ps_d = pd.enter_context(
                    tc.tile_pool(name="psd", bufs=3, space="PSUM"))

                y2acc = y2_p.tile([P, KD, 512], F32, tag="y2", name=_nm("y2"))
                for grp in range(4):
                    ht = ht_p.tile([P, 16, 512], BF16, tag="ht", name=_nm("ht"))
                    for fg in range(16):
                        m = grp * 16 + fg
                        wg_t = wgu_p.tile([P, KD, P], BF16, tag="wg",
                                          name=_nm("wg"))
                        nc.sync.dma_start(out=wg_t, in_=w_g.ap()[m])
                        wu_t = wgu_p.tile([P, KD, P], BF16, tag="wu",
                                          name=_nm("wu"))
                        nc.sync.dma_start(out=wu_t, in_=w_u.ap()[m])
                        gps = ps_gu.tile([P, 512], F32, tag="gu", name=_nm("gu"))
                        ups = ps_gu.tile([P, 512], F32, tag="gu", name=_nm("gu"))
                        for k in range(KD):
                            nc.tensor.matmul(gps[:], wg_t[:, k, :], h2[:, k, :],
                                             start=(k == 0), stop=(k == KD - 1))
                        for k in range(KD):
                            nc.tensor.matmul(ups[:], wu_t[:, k, :], h2[:, k, :],
                                             start=(k == 0), stop=(k == KD - 1))
                        sil = tmp3_p.tile([P, 512], BF16, tag="sil",
                                          name=_nm("sil"))
                        nc.scalar.activation(sil[:], gps[:], AF.Silu)
                        nc.vector.tensor_mul(ht[:, fg, :], sil[:], ups[:])
                    for mg in range(KD):
                        wd_t = wd_p.tile([P, 16, P], BF16, tag="wd",
                                         name=_nm("wd"))
                        nc.sync.dma_start(
                            out=wd_t,
                            in_=w_d.ap()[mg, :, grp * 16:(grp + 1) * 16, :])
                        dps = ps_d.tile([P, 512], F32, tag="d", name=_nm("d"))
                        for kk in range(16):
                            nc.tensor.matmul(dps[:], wd_t[:, kk, :],
                                             ht[:, kk, :],
                                             start=(kk == 0), stop=(kk == 15))
                        if grp == 0:
                            nc.vector.tensor_copy(y2acc[:, mg, :], dps[:])
                        else:
                            nc.vector.tensor_add(
                                y2acc[:, mg, :], y2acc[:, mg, :], dps[:])

                for m in range(KD):
                    o = tmp3_p.tile([P, 512], F32, tag="o", name=_nm("o"))
                    nc.vector.tensor_add(o[:], y2acc[:, m, :], yT[:, m, :])
                    nc.sync.dma_start(
                        out=outT.ap()[m * P:(m + 1) * P, :], in_=o)

    nc.compile()
    return nc


# ======================= host-side prep =======================

def _perm_matrix():
    perm = np.zeros((P, P), np.float32)
    for r in range(P):
        d = r % 64
        s = r + 32 if d < 32 else r - 32
        perm[s, r] = 1.0
    return perm


def _pack_weights(w_qkv, w_out, w_gate, w_up, w_down, w_norm1, w_norm2):
    """Shared (core-independent) packed weight arrays, bf16."""
    f32 = np.float32
    w_qkv = np.asarray(w_qkv, f32)
    n1 = np.asarray(w_norm1, f32)[:, None]
    n2 = np.asarray(w_norm2, f32)[:, None]
    wq = w_qkv[:, :2048] * n1
    wk = w_qkv[:, 2048:2560] * n1
    wv = w_qkv[:, 2560:3072] * n1
    wg = np.asarray(w_gate, f32) * n2
    wu = np.asarray(w_up, f32) * n2
    wo = np.asarray(w_out, f32)
    wd = np.asarray(w_down, f32)

    # Q column permutation: tile t=gp*4+j holds heads 8gp+j (parts 0:64)
    # and 8gp+4+j (parts 64:128)
    qcols = np.empty(2048, np.int64)
    for gp in range(4):
        for j in range(4):
            t = gp * 4 + j
            h1, h2 = 8 * gp + j, 8 * gp + 4 + j
            qcols[t * 128:t * 128 + 64] = np.arange(h1 * 64, h1 * 64 + 64)
            qcols[t * 128 + 64:(t + 1) * 128] = np.arange(h2 * 64, h2 * 64 + 64)
    wq_perm = wq[:, qcols]
    wo_perm = wo[qcols, :]   # ctx rows use the same layout

    def pack_out_k(w, n_m, n_k):  # [K, M] -> [m, p(k-row), k, c]
        K, M = w.shape
        assert K == n_k * P and M == n_m * P
        return np.ascontiguousarray(
            w.reshape(n_k, P, n_m, P).transpose(2, 1, 0, 3)).astype(BF)

    return {
        "w_q": pack_out_k(wq_perm, 16, KD),
        "w_k": np.ascontiguousarray(
            wk.reshape(KD, P, 512).transpose(1, 0, 2)).astype(BF),
        "w_v": np.ascontiguousarray(
            wv.reshape(KD, P, 512).transpose(1, 0, 2)).astype(BF),
        "w_o": pack_out_k(wo_perm, 16, KD),
        "w_g": pack_out_k(wg, 64, KD),
        "w_u": pack_out_k(wu, 64, KD),
        "w_d": pack_out_k(wd, 16, 64),
        "permM": _perm_matrix().astype(BF),
        "onesC": np.ones((P, 1), BF),
        "onesR": np.ones((1, P), BF),
    }


def _host_prep(c, x):
    """Per-core inputs: xT (bf16), xo32, rope tables, masks, biases."""
    f32 = np.float32
    if c <= 3:
        bA, chA = 0, c
        bB, chB = 1, 7 - c
    else:
        bA, chA = 1, 7 - c
        bB, chB = 0, c

    xTA = np.asarray(x[bA], f32).T  # [D, S]
    xTB = np.asarray(x[bB], f32).T

    xTc = np.zeros((D_MODEL, NCOL), dtype=f32)
    pos = np.zeros(NCOL, dtype=np.int64)
    oA, oB = chA * CHUNK, chB * CHUNK
    xTc[:, 0:256] = xTA[:, oA:oA + 256]
    pos[0:256] = np.arange(oA, oA + 256)
    xTc[:, 256:512] = xTB[:, oB:oB + 256]
    pos[256:512] = np.arange(oB, oB + 256)
    xTc[:, 512:512 + oA] = xTA[:, 0:oA]
    pos[512:512 + oA] = np.arange(oA)
    xTc[:, 1280:1280 + oB] = xTB[:, 0:oB]
    pos[1280:1280 + oB] = np.arange(oB)

    inv_freq = (ROPE_BASE ** (-np.arange(0, HEAD_DIM, 2, dtype=np.float64)
                              / HEAD_DIM))  # [32]
    ang = pos[None, :] * inv_freq[:, None]  # [32, NCOL]
    cos32 = np.cos(ang)
    sin32 = np.sin(ang)
    cosTc = np.empty((P, NCOL), dtype=f32)
    sinTc = np.empty((P, NCOL), dtype=f32)
    for hh in range(2):
        r = hh * 64
        cosTc[r:r + 32] = cos32
        cosTc[r + 32:r + 64] = cos32
        sinTc[r:r + 32] = -sin32
        sinTc[r + 32:r + 64] = sin32

    # causal masks for own-chunk diagonal tiles (kti 0/1), duplicated for
    # the two heads sharing a 512-wide score tile
    m = np.zeros((P, 2, 2, 256), dtype=f32)
    j = np.arange(256)[None, :]
    k_ = np.arange(P)[:, None]
    m[:, 0, 0] = m[:, 0, 1] = np.where(k_ > j, NEG, 0.0)
    m[:, 1, 0] = m[:, 1, 1] = np.where(k_ + P > j, NEG, 0.0)

    # exp bias: 0 for real key tiles, NEG for padding
    eb = np.zeros((2, 16), dtype=f32)
    for kti in range(2, 8):
        if (kti - 2) >= 2 * chA:
            eb[0, kti] = NEG
    for kti in range(2, 16):
        if (kti - 2) >= 2 * chB:
            eb[1, kti] = NEG
    ebc = np.broadcast_to(eb[None], (P, 2, 16)).copy()

    return {
        "xT": xTc.astype(BF),
        "xo32": np.ascontiguousarray(xTc[:, 0:512]),
        "cosT": cosTc.astype(BF),
        "sinT": sinTc.astype(BF),
        "mask2": m.astype(BF),
        "ebias": ebc,
    }


def run(inputs, trace=False):
    if "nc" not in _prog_cache:
        _prog_cache["nc"] = _build_program()
    nc = _prog_cache["nc"]
    from concourse.bass_utils import run_bass_kernel_spmd

    shared = _pack_weights(inputs["w_qkv"], inputs["w_out"], inputs["w_gate"],
                           inputs["w_up"], inputs["w_down"],
                           inputs["w_norm1"], inputs["w_norm2"])
    in_maps = []
    for c in range(N_CORES):
        mm = dict(shared)
        mm.update(_host_prep(c, inputs["x"]))
        in_maps.append(mm)
    res = run_bass_kernel_spmd(nc, in_maps, core_ids=list(range(N_CORES)),
                               trace=trace)

    out = np.empty((B, S, D_MODEL), dtype=np.float32)
    for c in range(N_CORES):
        oT = res.results[c]["outT"]  # [D, 512]
        if c <= 3:
            bA, chA = 0, c
            bB, chB = 1, 7 - c
        else:
            bA, chA = 1, 7 - c
            bB, chB = 0, c
        out[bA, chA * CHUNK:(chA + 1) * CHUNK] = oT[:, 0:256].T
        out[bB, chB * CHUNK:(chB + 1) * CHUNK] = oT[:, 256:512].T
    return out, res


def kernel(**inputs):
    out, _ = run(inputs, trace=False)
    return out
